# revision 13
# baseline (speedup 1.0000x reference)
"""Dual-codebook VQ (vq_codebook) Trainium2 kernel.

Strategy (per spec sharding hint): data-parallel over batch B=16 across 8
NeuronCores (2 batches/core), codebooks replicated. Per core, per stream:

  scores = x . e_k - 0.5|e_k|^2  via fp32r matmuls (lhsT = h's native [D, T]
  layout), bias applied during the DVE pass that also max-reduces per token;
  index extracted in one fused pass  sum((s == max) * iota);  z_q gathered by
  index via indirect DMA and transposed back to [D, T] on the PE.

  Losses use  sum(dist) = sum(h^2) - 2 * sum(max_score); per-core partial sums
  and per-token indices are returned and combined on the host (the all-reduce
  step of the data-parallel unshard).
"""

import json
import numpy as np

import concourse.bass as bass
import concourse.tile as tile
from concourse import mybir
from concourse.bass_utils import run_bass_kernel_spmd
from concourse.masks import make_identity

B, D, T, K = 16, 256, 2048, 1024
NCORES = 8
BPC = B // NCORES              # batches per core
NTILE_B = T // 128             # token tiles per batch  (16)
NTILE_S = BPC * NTILE_B        # token tiles per stream per core (32)
EPS = 1e-10

F32 = mybir.dt.float32
F32R = mybir.dt.float32r
F16 = mybir.dt.float16
I32 = mybir.dt.int32

_orig_to_json = None


def _install_bir_waitfix():
    """walrus rejects instructions with too many sync waits (4-slot limit on
    CTRL structs, tighter on the matmul LW struct). Split excess waits onto
    preceding NoOps on the same engine (same-engine program order makes this
    equivalent, just stricter)."""
    global _orig_to_json
    if _orig_to_json is not None:
        return
    _orig_to_json = bass.Bass.to_json_bytes

    def to_json_bytes_fixed(self, *a, **kw):
        raw = _orig_to_json(self, *a, **kw)
        d = json.loads(raw)
        n = [0]
        for f in d.get("functions", []):
            for blk in f.get("blocks", []):
                insts = blk.get("instructions")
                if not insts:
                    continue
                out = []
                for ins in insts:
                    si = ins.get("sync_info")
                    waits = (si or {}).get("on_wait") or []
                    op = ins.get("opcode", "")
                    if op == "ISA":
                        out.append(ins)
                        continue
                    lim = 1
                    if len(waits) > lim:
                        excess, keep = waits[:-lim], waits[-lim:]
                        for i in range(0, len(excess), 1):
                            n[0] += 1
                            out.append({
                                "name": f"I-wfix-{n[0]}",
                                "opcode": "NoOp",
                                "engine": ins["engine"],
                                "ins": [],
                                "outs": [],
                                "sync_info": {"on_wait": excess[i:i + 1],
                                              "on_update": []},
                            })
                        si["on_wait"] = keep
                    out.append(ins)
                blk["instructions"] = out
        return json.dumps(d).encode()

    bass.Bass.to_json_bytes = to_json_bytes_fixed


def _build_nc():
    nc = bass.Bass()
    h_in = [[nc.declare_dram_parameter(f"h{s}{p}", [BPC, D, T], F16, isOutput=False)
             for p in "ab"] for s in range(2)]
    cbt_in = [[nc.declare_dram_parameter(f"cbt{s}{p}", [D, K], F16, isOutput=False)
               for p in "ab"] for s in range(2)]
    cb_in = [nc.declare_dram_parameter(f"cb{s}", [K, D], F32, isOutput=False)
             for s in range(2)]
    bias_in = [nc.declare_dram_parameter(f"bias{s}", [2, K], F16, isOutput=False)
               for s in range(2)]
    iota_in = nc.declare_dram_parameter("iota", [128, K], F32, isOutput=False)
    z_out = [nc.declare_dram_parameter(f"z{s}", [BPC, D, T], F32, isOutput=True)
             for s in range(2)]
    idx_out = nc.declare_dram_parameter("idx", [2, 128, NTILE_S], F32, isOutput=True)
    sums_out = nc.declare_dram_parameter("sums", [128, 4], F32, isOutput=True)

    with tile.TileContext(nc) as tc:
        with (
            tc.tile_pool(name="const", bufs=1) as cpool,
            tc.tile_pool(name="hbuf", bufs=2) as hpool,
            tc.tile_pool(name="sc", bufs=4) as scpool,
            tc.tile_pool(name="small", bufs=8) as smpool,
            tc.tile_pool(name="gath", bufs=6) as gpool,
            tc.tile_pool(name="acc", bufs=1) as apool,
            tc.tile_pool(name="ps", bufs=6, space="PSUM") as pspool,
            tc.tile_pool(name="pst", bufs=2, space="PSUM") as pstpool,
        ):
            # constants
            cbt = [[[cpool.tile([128, K], F16, tag=f"cbt{s}{p}{dh}", name=f"cbt{s}{p}{dh}")
                     for dh in range(2)] for p in range(2)] for s in range(2)]
            biasb = [cpool.tile([2, K], F16, tag=f"bias{s}", name=f"biasb{s}") for s in range(2)]
            ones2 = cpool.tile([2, 128], F16, tag="ones2", name="ones2")
            nc.vector.memset(ones2[:], 1.0)
            iota = cpool.tile([128, K], F32, tag="iota", name="iotat")
            ident = cpool.tile([128, 128], F32, tag="ident", name="ident")
            make_identity(nc, ident[:])
            for s in range(2):
                for p in range(2):
                    for dh in range(2):
                        nc.sync.dma_start(cbt[s][p][dh][:],
                                          cbt_in[s][p][dh * 128:(dh + 1) * 128, :])
                nc.sync.dma_start(biasb[s][:], bias_in[s][:])
            nc.sync.dma_start(iota[:], iota_in[:])

            # accumulators: col 0/1 = sum(max) top/bot, 2/3 = sum(h^2) top/bot
            accs = apool.tile([128, 4], F32, tag="accs", name="accs")
            nc.vector.memset(accs[:], 0.0)
            idx_stash = apool.tile([128, 2 * NTILE_S], F32, tag="idxstash", name="idxstash")

            hsq_junk = apool.tile([128, T], mybir.dt.bfloat16, tag="hsqjunk", name="hsqjunk")

            for b in range(BPC):
                for s in range(2):
                    hda = hpool.tile([128, 2 * T], F16, tag="hda", name="hda")
                    hdb = hpool.tile([128, 2 * T], F16, tag="hdb", name="hdb")
                    for p, hp in ((0, hda), (1, hdb)):
                        nc.sync.dma_start(hp[:, 0:T], h_in[s][p][b, 0:128, :])
                        nc.sync.dma_start(hp[:, T:2 * T], h_in[s][p][b, 128:256, :])

                    # sum(h^2) via h_sum = hi + lo (== h to 2^-22)
                    for dh in range(2):
                        hsum = hpool.tile([128, T], F32, tag="hsum", name="hsum")
                        nc.vector.tensor_add(hsum[:], hda[:, dh * T:(dh + 1) * T],
                                             hdb[:, dh * T:(dh + 1) * T])
                        hcol = smpool.tile([128, 1], F32, tag="hcol", name="hcol")
                        nc.scalar.activation(
                            hsq_junk[:], hsum[:],
                            mybir.ActivationFunctionType.Square,
                            accum_out=hcol[:])
                        nc.vector.tensor_add(accs[:, 2 + s:3 + s], accs[:, 2 + s:3 + s], hcol[:])

                    for t in range(NTILE_B):
                        tg = b * NTILE_B + t          # tile index within stream
                        sl = slice(t * 128, (t + 1) * 128)
                        ps = [pspool.tile([128, 512], F32, tag="ps", name="pstile") for _ in range(2)]
                        for bank in range(2):
                            ksl = slice(bank * 512, (bank + 1) * 512)
                            prods = [(hda, 0), (hdb, 0), (hda, 1)]  # aa, ba, ab
                            nmm = 0
                            for hp, cp in prods:
                                for dh in range(2):
                                    nc.tensor.matmul(
                                        ps[bank][:],
                                        hp[:, dh * T + t * 128:dh * T + (t + 1) * 128],
                                        cbt[s][cp][dh][:, ksl],
                                        start=(nmm == 0), stop=False)
                                    nmm += 1
                            nc.tensor.matmul(ps[bank][:], ones2[:],
                                             biasb[s][:, ksl], start=False, stop=True)

                        mx = smpool.tile([128, 2], F32, tag="mx", name="mx")
                        for bank in range(2):
                            nc.vector.reduce_max(mx[:, bank:bank + 1], ps[bank][:],
                                                 axis=mybir.AxisListType.X)
                        mxf = smpool.tile([128, 1], F32, tag="mxf", name="mxf")
                        nc.vector.tensor_tensor(mxf[:], mx[:, 0:1], mx[:, 1:2],
                                                op=mybir.AluOpType.max)
                        junk = scpool.tile([128, 512], F32, tag="junk", name="junkt")
                        idxp = smpool.tile([128, 2], F32, tag="idxp", name="idxp")
                        for bank in range(2):
                            ksl = slice(bank * 512, (bank + 1) * 512)
                            nc.vector.scalar_tensor_tensor(
                                out=junk[:], in0=ps[bank][:], scalar=mxf[:],
                                in1=iota[:, ksl],
                                op0=mybir.AluOpType.is_equal, op1=mybir.AluOpType.mult,
                                accum_out=idxp[:, bank:bank + 1])
                        idxf = smpool.tile([128, 1], F32, tag="idxf", name="idxf")
                        nc.vector.tensor_add(idxf[:], idxp[:, 0:1], idxp[:, 1:2])
                        # bookkeeping
                        nc.vector.tensor_add(accs[:, s:s + 1], accs[:, s:s + 1], mxf[:])
                        nc.scalar.copy(idx_stash[:, s * NTILE_S + tg:s * NTILE_S + tg + 1], idxf[:])
                        idxc = smpool.tile([128, 1], F32, tag="idxc", name="idxc")
                        nc.vector.tensor_scalar(
                            out=idxc[:], in0=idxf[:], scalar1=float(K - 1), scalar2=0.0,
                            op0=mybir.AluOpType.min, op1=mybir.AluOpType.max)
                        idxi = smpool.tile([128, 1], I32, tag="idxi", name="idxi")
                        nc.scalar.copy(idxi[:], idxc[:])

                        # gather codebook rows -> [tokens, 256]
                        zq = gpool.tile([128, D], F32, tag="zq", name="zqtile")
                        nc.gpsimd.indirect_dma_start(
                            out=zq[:], out_offset=None, in_=cb_in[s][:],
                            in_offset=bass.IndirectOffsetOnAxis(ap=idxi[:, :1], axis=0))
                        # transpose to [D, tokens] and store
                        pst = pstpool.tile([128, 256], F32, tag="pst", name="psttile")
                        zqt = gpool.tile([128, 256], F32, tag="zqt", name="zqttile")
                        for dh in range(2):
                            nc.tensor.transpose(pst[:, dh * 128:(dh + 1) * 128],
                                                zq[:, dh * 128:(dh + 1) * 128], ident[:])
                        nc.scalar.copy(zqt[:], pst[:])
                        for dh in range(2):
                            nc.sync.dma_start(
                                z_out[s][b, dh * 128:(dh + 1) * 128, sl],
                                zqt[:, dh * 128:(dh + 1) * 128])

            nc.sync.dma_start(idx_out[0], idx_stash[:, 0:NTILE_S])
            nc.sync.dma_start(idx_out[1], idx_stash[:, NTILE_S:2 * NTILE_S])
            nc.sync.dma_start(sums_out[:], accs[:])
    return nc


_NC_CACHE = None
LAST_EXEC_NS = None


def estimate_exec_ns():
    """Cost-model (TimelineSim) per-core exec estimate; used when the axon
    NTFF profiling hook is unavailable in this container."""
    _install_bir_waitfix()
    global _NC_CACHE
    if _NC_CACHE is None:
        _NC_CACHE = _build_nc()
    from concourse.timeline_sim import TimelineSim
    return TimelineSim(_NC_CACHE, no_exec=True).simulate()


def kernel(h_top, h_bot, codebook_top, codebook_bot):
    global _NC_CACHE
    _install_bir_waitfix()
    if _NC_CACHE is None:
        _NC_CACHE = _build_nc()
    nc = _NC_CACHE

    h = [np.ascontiguousarray(h_top, dtype=np.float32),
         np.ascontiguousarray(h_bot, dtype=np.float32)]
    cb = [np.ascontiguousarray(codebook_top, dtype=np.float32),
          np.ascontiguousarray(codebook_bot, dtype=np.float32)]
    ha = [x.astype(np.float16) for x in h]
    hb = [(x - a.astype(np.float32)).astype(np.float16) for x, a in zip(h, ha)]
    cbt_f = [np.ascontiguousarray(c.T) for c in cb]
    cbta = [c.astype(np.float16) for c in cbt_f]
    cbtb = [(c - a.astype(np.float32)).astype(np.float16) for c, a in zip(cbt_f, cbta)]
    bias = []
    for c in cb:
        bf = (-0.5 * (c.astype(np.float64) ** 2).sum(-1)).astype(np.float32)
        bh = bf.astype(np.float16)
        bl = (bf - bh.astype(np.float32)).astype(np.float16)
        bias.append(np.stack([bh, bl]))
    iota = np.broadcast_to(np.arange(K, dtype=np.float32), (128, K)).copy()

    in_maps = []
    for c in range(NCORES):
        bsl = slice(c * BPC, (c + 1) * BPC)
        in_maps.append({
            "h0a": ha[0][bsl], "h0b": hb[0][bsl],
            "h1a": ha[1][bsl], "h1b": hb[1][bsl],
            "cbt0a": cbta[0], "cbt0b": cbtb[0],
            "cbt1a": cbta[1], "cbt1b": cbtb[1],
            "cb0": cb[0], "cb1": cb[1],
            "bias0": bias[0], "bias1": bias[1],
            "iota": iota,
        })
    import os
    trace = os.environ.get("VQ_TRACE", "0") == "1"
    res = run_bass_kernel_spmd(nc, in_maps, core_ids=list(range(NCORES)), trace=trace)
    global LAST_EXEC_NS
    LAST_EXEC_NS = res.exec_time_ns

    z = [np.empty((B, D, T), np.float32) for _ in range(2)]
    idx_all = [[], []]
    sums = np.zeros(4, np.float64)
    for c, r in enumerate(res.results):
        for s in range(2):
            z[s][c * BPC:(c + 1) * BPC] = r[f"z{s}"]
            idx_all[s].append(r["idx"][s].astype(np.int64).ravel())
        sums += r["sums"].astype(np.float64).sum(axis=0)

    n_el = float(B * D * T)
    loss = np.float32((sums[2] - 2.0 * sums[0]) / n_el + (sums[3] - 2.0 * sums[1]) / n_el)
    perp = []
    for s in range(2):
        counts = np.bincount(np.concatenate(idx_all[s]), minlength=K).astype(np.float32)
        p = counts / np.float32(B * T)
        perp.append(np.float32(np.exp(-np.sum(p * np.log(p + np.float32(EPS))))))
    return (z[0], z[1], loss, loss.copy(), perp[0], perp[1])
